# revision 4
# baseline (speedup 1.0000x reference)
"""Multi-head self-attention (B=4, S=2048, D=512, H=8, d=64) on 8 trn2 cores.

Sharding: 2 cores per batch element; each core computes 4 heads (a 256-wide
column slice of Wq/Wk/Wv and row slice of Wh) and produces a partial
[S, 512] output; the host sums the two partials per batch and adds bh.

v2: the HW microbenchmarks showed ScalarE's exp stream (~1.0us per
[128,1024] exp) is the true floor and PE has large real slack (pipelined
LDWEIGHTS->MATMUL streams run ~1.6x the cost model).  The v1 kernel lost
~120us to the single-bank ps_x rotator: every projection group serialized
PE -> DVE -> PE on one PSUM bank, blocking the in-order PE queue and
starving the exp stream.  v2:
  - ps_x bufs=2 (double-buffered rotator) by shrinking ps_av to 2 bufs;
  - filler groups are transient (one step: all 4 K-chunks + DVE drain),
    so each group occupies one rotator slot briefly and alternates banks;
  - V projection runs through the rotator too ([128,512] tiles), keeping
    ps_sc exclusively for score tiles (clean double-buffering for exp);
  - softmax normalization (recip -> K=1 broadcast -> attnT=av*bc) is
    emitted at quarter boundaries (program order before the next av
    allocations, required by av bufs=2);
  - quarter (0,0) runs just-in-time V tiles + the kT/qT groups needed
    within the quarter; everything else is pump-budgeted fillers.
"""

import numpy as np

NUM_HEADS = 8
D_MODEL = 512
D_HEAD = 64
B = 4
S = 2048
H_PER_CORE = 4          # heads per core
DQ = H_PER_CORE * D_HEAD  # 256 = per-core q/k/v width
N_CORES = 8
SCALE = 1.0 / np.sqrt(D_HEAD)

_KO = D_MODEL // 128    # 4 contraction chunks for the projections
_NT = S // 128          # 16 tiles of 128 along S
_VW = D_HEAD + 1        # 65: v columns per head incl. ones column


def _split_excess_waits(nc):
    """Walrus's TRN2 codegen fits very few sync-waits per instruction (one on
    a Matmult's weight-load, few on drains).  Move excess waits onto NoOps
    inserted just before the instruction — engine queues are in-order, so a
    wait on a preceding same-engine instruction still protects it."""
    import concourse.mybir as mybir

    n_fixed = 0
    for f in nc.m.functions:
        for bb in f.blocks:
            insts = list(bb.instructions)
            out = []
            changed = False
            for ins in insts:
                si = ins.sync_info
                if si is not None and si.on_wait and len(si.on_wait) > 1:
                    waits = list(si.on_wait)
                    # An exp/matmul waiting on its OWN engine's completion sem
                    # is a slot-recycle WAW guard: implied by in-order issue,
                    # with the interleaved cross-engine reader guarded by the
                    # remaining wait.  Dropping it avoids a NoOp on the
                    # bottleneck queue (one per exp otherwise).
                    if isinstance(ins, (mybir.InstActivation, mybir.InstMatmult)):
                        eng_pfx = str(ins.engine).split(".")[-1] + "_"
                        cross = [w for w in waits
                                 if not str(getattr(w, "ant_name", "")).startswith(eng_pfx)]
                        if cross and len(cross) < len(waits):
                            waits = cross
                    for j, w in enumerate(waits[1:]):
                        nop = mybir.InstNoOp(
                            name=f"{ins.name}_waitnop{j}", ins=[], outs=[])
                        nop.engine = ins.engine
                        nop.sync_info = mybir.SyncInfo(on_wait=[w], on_update=[])
                        out.append(nop)
                    ins.sync_info = mybir.SyncInfo(
                        on_wait=waits[:1], on_update=list(si.on_update or []))
                    n_fixed += 1
                    changed = True
                out.append(ins)
            if changed:
                bb.instructions = out
    return n_fixed


def build_nc(nrep=1):
    """Build the per-core Bass program.  nrep>1 repeats the compute body
    (same tiles, idempotent) for wall-clock timing amplification."""
    from collections import deque

    import concourse.bass as bass
    import concourse.mybir as mybir
    import concourse.tile as tile

    f32 = mybir.dt.float32
    f32r = mybir.dt.float32r
    bf16 = mybir.dt.bfloat16
    AF = mybir.ActivationFunctionType

    nc = bass.Bass()
    x_d = nc.dram_tensor("x", [D_MODEL, S], bf16, kind="ExternalInput")
    wq_d = nc.dram_tensor("wq", [D_MODEL, DQ], bf16, kind="ExternalInput")
    wk_d = nc.dram_tensor("wk", [D_MODEL, DQ], bf16, kind="ExternalInput")
    wv_d = nc.dram_tensor("wv", [D_MODEL, DQ], bf16, kind="ExternalInput")
    wh_d = nc.dram_tensor("wh", [DQ, D_MODEL], bf16, kind="ExternalInput")
    bq_d = nc.dram_tensor("bq", [DQ], f32, kind="ExternalInput")
    bk_d = nc.dram_tensor("bk", [DQ], f32, kind="ExternalInput")
    bv_d = nc.dram_tensor("bv", [DQ], f32, kind="ExternalInput")
    o_d = nc.dram_tensor("o", [S, D_MODEL], bf16, kind="ExternalOutput")

    with (
        nc.allow_low_precision(reason="float32r attention pipeline"),
        tile.TileContext(nc) as tc,
        tc.tile_pool(name="cst", bufs=1) as cst,
        tc.tile_pool(name="big", bufs=1) as big,
        tc.tile_pool(name="pr", bufs=5) as pr,
        tc.tile_pool(name="ps_sc", bufs=2, space="PSUM") as ps_sc,
        tc.tile_pool(name="ps_av", bufs=2, space="PSUM") as ps_av,
        tc.tile_pool(name="ps_x", bufs=2, space="PSUM") as ps_x,
    ):
        # ones_col [1,64]: K=1 matmul broadcasts a [1,512] row across 64
        # output partitions (denominator broadcast + PE warmup fodder)
        ones_col = cst.tile([1, D_HEAD], f32r)
        nc.gpsimd.memset(ones_col.bitcast(f32)[:], 1.0)

        # ---- DMA schedule: ONE ring (SP) in exact consumption order, so
        # the shared DMA pipe never reorders a late-needed transfer ahead of
        # an early-needed one: wk, x01, bk, wq, x23, bq, wv, x4..x7, bv, wh.
        w_sb = {}
        bias_sb = {}
        xT = big.tile([128, _KO, S], bf16, tag="xT")
        x_r = x_d.rearrange("(a p) s -> p a s", p=128)

        def _load_w(name, dram, shp):
            wt = big.tile(list(shp), bf16, tag=f"w_{name}")
            nc.sync.dma_start(wt[:], dram.rearrange("(a p) m -> p a m", p=128))
            w_sb[name] = wt

        def _load_b(name, dram):
            bt = cst.tile([128, DQ // 128], f32, tag=f"b_{name}")
            nc.sync.dma_start(bt[:], dram.rearrange("(o p) -> p o", p=128))
            bias_sb[name] = bt

        def _load_x(sg):
            nc.sync.dma_start(
                xT[:, :, sg * 256:(sg + 1) * 256],
                x_r[:, :, sg * 256:(sg + 1) * 256])

        _load_w("wk", wk_d, (128, _KO, DQ))
        _load_x(0); _load_x(1)
        _load_b("bk", bk_d)
        _load_w("wq", wq_d, (128, _KO, DQ))
        _load_b("bq", bq_d)
        _load_w("wv", wv_d, (128, _KO, DQ))
        _load_b("bv", bv_d)
        _load_x(2); _load_x(3); _load_x(4); _load_x(5); _load_x(6); _load_x(7)
        _load_w("wh", wh_d, (128, DQ // 128, D_MODEL))

        for _rep in range(nrep):
            qT = big.tile([128, DQ // 128, S], bf16, tag="qT")
            kT = big.tile([128, DQ // 128, S], bf16, tag="kT")
            attnT = big.tile([128, DQ // 128, S], bf16, tag="attnT")
            v_aug = big.tile([128, _NT, H_PER_CORE * _VW], bf16, tag="v_aug")
            # only the per-head ones columns need (re)setting; pv moves
            # overwrite the rest every rep
            nc.gpsimd.memset(
                v_aug.rearrange(
                    "p t (h w) -> p t h w", w=_VW)[:, :, :, D_HEAD:], 1.0)

            # ---------- projection helpers ----------
            def proj_group(dst, wname, bname, o, sg, pool, tag="sc"):
                """dst[:, o, sg*512:(sg+1)*512] = (W^T x + b), via 4 K-chunks
                accumulated in one PSUM tile, drained by DVE (bias add)."""
                p = pool.tile([128, 512], f32, tag=tag, name="p_qk")
                for ko in range(_KO):
                    nc.tensor.matmul(
                        p[:],
                        w_sb[wname][:, ko, o * 128:(o + 1) * 128],
                        xT[:, ko, sg * 512:(sg + 1) * 512],
                        start=(ko == 0), stop=(ko == _KO - 1))
                nc.vector.tensor_scalar_add(
                    dst[:, o, sg * 512:(sg + 1) * 512], p[:],
                    bias_sb[bname][:, o:o + 1])

            def project_v(t, pool, tag="px"):
                pv = pool.tile([128, 512], f32, tag=tag, name="pv")
                for ko in range(_KO):
                    nc.tensor.matmul(
                        pv[:, :DQ],
                        xT[:, ko, t * 128:(t + 1) * 128],
                        w_sb["wv"][:, ko, :],
                        start=(ko == 0), stop=(ko == _KO - 1))
                nc.vector.tensor_copy(
                    v_aug[:, t, :].rearrange(
                        "p (h w) -> p h w", w=_VW)[:, :, :D_HEAD],
                    pv[:, :DQ].rearrange("p (h w) -> p h w", w=D_HEAD))

            # ---------- filler machinery ----------
            fillers = deque()   # items: (pe_rows, closure)
            spent = [0.0]
            clock = [0.0]       # steps elapsed (1 step = one exp)

            def pump(budget_rows_per_step):
                clock[0] += 1.0
                while fillers and spent[0] < clock[0] * budget_rows_per_step:
                    rows, go = fillers.popleft()
                    go()
                    spent[0] += rows

            def drain_fillers():
                while fillers:
                    rows, go = fillers.popleft()
                    go()
                    spent[0] += rows

            # ---------- attention ----------
            def normalize_boundary(o, jq):
                """invZ (DVE) -> K=1 broadcast per head (PE, px rotator) ->
                attnT = av * bc (DVE).  Emitted at t==0 of the NEXT quarter
                (after its first scores+exp), so the av-slot WAR readers are
                in program order before the next av-pair's first write
                (required by av bufs=2) while the PE bc matmuls hide behind
                the first exp."""
                sq = jq * 512
                av0, av1 = cur_av.pop((o, jq))
                iz0 = pr.tile([1, 512], f32r, tag="invz")
                iz1 = pr.tile([1, 512], f32r, tag="invz")
                nc.vector.reciprocal(iz0[:], av0[D_HEAD:_VW, :])
                nc.vector.reciprocal(iz1[:], av1[D_HEAD:_VW, :])
                bc = ps_x.tile([128, 512], f32, tag="px", name="bc0")
                nc.tensor.matmul(bc[0:D_HEAD, :], ones_col[:],
                                 iz0[:], start=True, stop=True)
                bc_sb = pr.tile([128, 512], f32, tag="bcsb")
                nc.vector.tensor_copy(bc_sb[0:64, :], bc[0:D_HEAD, :])
                nc.vector.tensor_tensor(
                    attnT[0:64, o, sq:sq + 512],
                    av0[0:D_HEAD, :], bc_sb[0:64, :],
                    mybir.AluOpType.mult)
                bc1 = ps_x.tile([128, 512], f32, tag="px", name="bc1")
                nc.tensor.matmul(bc1[0:D_HEAD, :], ones_col[:],
                                 iz1[:], start=True, stop=True)
                nc.vector.tensor_copy(bc_sb[64:128, :], bc1[0:D_HEAD, :])
                nc.vector.tensor_tensor(
                    attnT[64:128, o, sq:sq + 512],
                    av1[0:D_HEAD, :], bc_sb[64:128, :],
                    mybir.AluOpType.mult)

            cur_av = {}
            pend = deque()   # cross-quarter: (emit_fn, t, probs)

            def attend_quarter(o, jq, budget, lag=3, step_queue=None,
                               boundary=None):
                h0, h1 = 2 * o, 2 * o + 1
                sq = jq * 512
                box = {}

                def emit_av(t, probs):
                    if t == 0:
                        # lazy: allocate after the previous pair's readers
                        # (normalize_boundary) are emitted — av bufs=2
                        box["av"] = (
                            ps_av.tile([128, 512], f32, tag="av", name="av0"),
                            ps_av.tile([128, 512], f32, tag="av", name="av1"))
                        cur_av[(o, jq)] = box["av"]
                    av0, av1 = box["av"]
                    nc.tensor.matmul(
                        av0[0:_VW, :],
                        v_aug[:, t, h0 * _VW:(h0 + 1) * _VW],
                        probs[:, 0:512],
                        start=(t == 0), stop=(t == _NT - 1))
                    nc.tensor.matmul(
                        av1[0:_VW, :],
                        v_aug[:, t, h1 * _VW:(h1 + 1) * _VW],
                        probs[:, 512:1024],
                        start=(t == 0), stop=(t == _NT - 1))

                for t in range(_NT):
                    sc = ps_sc.tile([128, 1024], f32, tag="sc", name="sc")
                    nc.tensor.matmul(
                        sc[:, 0:512],
                        kT[0:64, o, t * 128:(t + 1) * 128],
                        qT[0:64, o, sq:sq + 512],
                        start=True, stop=True)
                    nc.tensor.matmul(
                        sc[:, 512:1024],
                        kT[64:128, o, t * 128:(t + 1) * 128],
                        qT[64:128, o, sq:sq + 512],
                        start=True, stop=True)
                    probs = pr.tile([128, 1024], bf16, tag="probs")
                    nc.scalar.activation(probs[:], sc[:], AF.Exp,
                                         scale=float(SCALE))
                    if t == 0:
                        # drain the previous quarter's pending avs right
                        # after this quarter's first scores+exp, so the PE
                        # stream never bubbles at the boundary; then emit the
                        # previous quarter's normalization BEFORE any pump
                        # (out_sg fillers read the attnT it writes) and
                        # before this quarter's first av allocation (pop at
                        # t==lag, av bufs=2)
                        while pend:
                            fn, t_, p_ = pend.popleft()
                            fn(t_, p_)
                        if boundary is not None:
                            boundary()
                    elif len(pend) >= lag:
                        fn, t_, p_ = pend.popleft()
                        fn(t_, p_)
                    pend.append((emit_av, t, probs))
                    if step_queue:
                        for go in step_queue.popleft():
                            go()
                        clock[0] += 1.0
                    else:
                        pump(budget)

            # ---------- output projection ----------
            def out_sg(sg, pool):
                """out rows [sg*128, sg*128+128) = attnT^T @ Wh, 2 K-chunks
                in one transient rotator tile."""
                po = pool.tile([128, 512], f32,
                               tag="px" if pool is ps_x else "av", name="po")
                for o in range(DQ // 128):
                    nc.tensor.matmul(
                        po[:],
                        attnT[:, o, sg * 128:(sg + 1) * 128],
                        w_sb["wh"][:, o, :],
                        start=(o == 0), stop=(o == DQ // 128 - 1))
                ot = pr.tile([128, 512], bf16, tag="ot")
                nc.vector.tensor_copy(ot[:], po[:])
                nc.sync.dma_start(
                    o_d.rearrange("(t p) d -> p t d", p=128)[:, sg, :],
                    ot[:])

            # ================= schedule =================
            # PE p-state warmup: tiny K=2 matmuls into the rotator bank keep
            # PE continuously busy from ~0.6us so the 2.4GHz ramp completes
            # before the real projections start.
            if _rep == 0:
                warm = ps_x.tile([128, 512], f32, tag="px", name="warm")
                for _ in range(30):
                    nc.tensor.matmul(warm[0:D_HEAD, 0:D_HEAD], ones_col[:],
                                     ones_col[:], start=True, stop=True)

            # pre-phase: only what exp step 0..3 strictly needs
            proj_group(kT, "wk", "bk", 0, 0, ps_x, tag="px")
            proj_group(qT, "wq", "bq", 0, 0, ps_x, tag="px")
            project_v(0, ps_x)
            project_v(1, ps_x)

            # quarter (0,0) mandatory per-step work, all just-in-time:
            # one V tile per step plus the kT sg1..3 / qT sg1 groups at
            # steps 1,5,9,13 (kT sg k must land before scores step 4k).
            jq0_groups = deque([
                lambda: proj_group(kT, "wk", "bk", 0, 1, ps_x, tag="px"),
                lambda: proj_group(kT, "wk", "bk", 0, 2, ps_x, tag="px"),
                lambda: proj_group(kT, "wk", "bk", 0, 3, ps_x, tag="px"),
                lambda: proj_group(qT, "wq", "bq", 0, 1, ps_x, tag="px"),
            ])
            step_queue = deque()
            for t in range(_NT):
                items = []
                if t + 2 < _NT:
                    items.append(lambda t=t: project_v(t + 2, ps_x))
                if t % 4 == 1 and jq0_groups:
                    items.append(jq0_groups.popleft())
                step_queue.append(items)

            # fillers for the rest: whole transient groups (2048 rows each)
            for sg in (2, 3):
                fillers.append((2048, lambda sg=sg: proj_group(
                    qT, "wq", "bq", 0, sg, ps_x, tag="px")))
            for sg in range(4):
                fillers.append((2048, lambda sg=sg: proj_group(
                    kT, "wk", "bk", 1, sg, ps_x, tag="px")))
            for sg in range(4):
                fillers.append((2048, lambda sg=sg: proj_group(
                    qT, "wq", "bq", 1, sg, ps_x, tag="px")))

            BUDGET = 700  # PE filler rows per exp-step of slack
            attend_quarter(0, 0, BUDGET, step_queue=step_queue)
            # quarter (0,0) advanced the clock without spending filler
            # allowance; reset so the pump doesn't flood PE at step 17
            spent[0] = clock[0] * BUDGET
            for jq in range(1, 4):
                attend_quarter(0, jq, BUDGET,
                               boundary=lambda jq=jq: normalize_boundary(
                                   0, jq - 1))
            for jq in range(4):
                if jq > 0:
                    for sg in range(4 * (jq - 1), 4 * jq):
                        fillers.append((1024, lambda sg=sg: out_sg(sg, ps_x)))
                bnd = (lambda: normalize_boundary(0, 3)) if jq == 0 else (
                    lambda jq=jq: normalize_boundary(1, jq - 1))
                attend_quarter(1, jq, BUDGET, boundary=bnd)
            while pend:
                fn, t_, p_ = pend.popleft()
                fn(t_, p_)
            normalize_boundary(1, 3)
            # tail: remaining fillers, then the final output quarter using
            # both the rotator and the (now idle) av pool banks
            drain_fillers()
            for sg in range(12, 16):
                out_sg(sg, ps_av if sg % 2 else ps_x)

    _split_excess_waits(nc)
    return nc


def _in_maps(inputs):
    import ml_dtypes
    x = np.ascontiguousarray(np.asarray(inputs["x"], dtype=np.float32))
    maps = []
    for c in range(N_CORES):
        b, g = c // 2, c % 2
        hs = slice(g * DQ, (g + 1) * DQ)
        maps.append({
            "x": np.ascontiguousarray(x[b].T.astype(ml_dtypes.bfloat16)),
            "wq": np.ascontiguousarray(np.asarray(inputs["Wq"], np.float32)[:, hs].astype(ml_dtypes.bfloat16)),
            "wk": np.ascontiguousarray(np.asarray(inputs["Wk"], np.float32)[:, hs].astype(ml_dtypes.bfloat16)),
            "wv": np.ascontiguousarray(np.asarray(inputs["Wv"], np.float32)[:, hs].astype(ml_dtypes.bfloat16)),
            "wh": np.ascontiguousarray(np.asarray(inputs["Wh"], np.float32)[hs, :].astype(ml_dtypes.bfloat16)),
            "bq": np.ascontiguousarray(np.asarray(inputs["bq"], np.float32)[hs]),
            "bk": np.ascontiguousarray(np.asarray(inputs["bk"], np.float32)[hs]),
            "bv": np.ascontiguousarray(np.asarray(inputs["bv"], np.float32)[hs]),
        })
    return maps


def kernel(**inputs):
    from concourse.bass_utils import run_bass_kernel_spmd

    nc = build_nc(nrep=1)
    maps = _in_maps(inputs)
    res = run_bass_kernel_spmd(nc, maps, core_ids=list(range(N_CORES)))
    bh = (np.asarray(inputs["bh"], np.float32)
          + np.asarray(inputs["bv"], np.float32)
          @ np.asarray(inputs["Wh"], np.float32))
    out = np.empty((B, S, D_MODEL), np.float32)
    for b in range(B):
        out[b] = (res.results[2 * b]["o"].astype(np.float32)
                  + res.results[2 * b + 1]["o"].astype(np.float32) + bh)
    return out
